# revision 12
# baseline (speedup 1.0000x reference)
"""Chamfer loss Trainium2 kernel.

Per-sample Chamfer loss over (bs=8, n=4096, d=3) point clouds, data-parallel
over the batch axis: one sample per NeuronCore, no cross-core communication.

Math: dist[i,j] = sqrt(eps + relu(||y_i||^2 + ||x_j||^2 - 2 y_i.x_j)).
sqrt(eps + relu(.)) is monotonic, so min-reduce the *squared* matrix and apply
the transform to the reduced 4096-vectors only.

The squared-distance matrix is produced on the TensorEngine as a single K=18
bf16 matmul per tile: y/x are split hi+lo in bf16 (y ~ y0+y1), the squared
norms into three bf16 addends, and all product terms are stacked along the
contraction axis. PSUM accumulates in fp32, giving |sq - exact| ~ 3e-4, i.e.
~1e-5 relative error on the final loss. bf16 streams 1 cycle/row vs fp32's 4.

Per 128-row block (32 of them):
  PE    : 8 matmuls (N=512) into two [128,2048] fp32 PSUM tiles
  ACT   : copies each PSUM tile to an SBUF bf16 strip (frees PSUM, enables
          2x/4x-rate bf16 DVE ops)
  DVE   : running column-min (elementwise bf16 tensor_tensor min into
          colacc[128,4096], 2x mode) and row-min via tensor_scalar with
          min-accumulate (4x mode, one op per strip)
Epilogue: colacc partition-min via PE transpose + DVE min-reduce per 128-col
chunk, then relu/+eps/sqrt on the two [128,32] min matrices, sum-reduce, a
ones-vector matmul for the partition sum, scale by 1/4096.
"""

import os
import sys
import functools

for _p in ("/opt/trn_rl_repo", "/root/.axon_site/_ro/trn_rl_repo"):
    if os.path.isdir(_p) and _p not in sys.path:
        sys.path.insert(0, _p)

import numpy as np
import ml_dtypes

import concourse.bass as bass
import concourse.bacc as bacc
import concourse.mybir as mybir
import concourse.tile as tile
from concourse import bass_utils

BF16 = ml_dtypes.bfloat16
F32 = np.float32

N = 4096          # points per cloud
P = 128           # partitions
NB = N // P       # 32 row blocks
H = 2048          # strip width (half of N), 4 PSUM banks
K = 24            # stacked contraction rows
EPS = 1e-6
BIG = 1e30

AF = mybir.ActivationFunctionType
ALU = mybir.AluOpType
AX = mybir.AxisListType
DT = mybir.dt




def _emit(nc):
    lhsT_d = nc.dram_tensor("lhst_in", [K, N], DT.bfloat16, kind="ExternalInput")
    rhs_d = nc.dram_tensor("rhs_in", [K, N], DT.bfloat16, kind="ExternalInput")
    ident_d = nc.dram_tensor("ident_in", [P, P], DT.bfloat16, kind="ExternalInput")
    out_d = nc.dram_tensor("loss_out", [1, 1], DT.float32, kind="ExternalOutput")

    with tile.TileContext(nc) as tc:
        with (
            tc.tile_pool(name="const", bufs=1) as cpool,
            tc.tile_pool(name="strip", bufs=4) as spool,
            tc.tile_pool(name="scr", bufs=2) as scrpool,
            tc.tile_pool(name="psum", bufs=2, space="PSUM") as ppool,
        ):
            lhsT = cpool.tile([K, N], DT.bfloat16, tag="lhsT")
            rhs = cpool.tile([K, N], DT.bfloat16, tag="rhs")
            ident = cpool.tile([P, P], DT.bfloat16, tag="ident")
            colacc = cpool.tile([P, N], DT.bfloat16, tag="colacc")
            # bf16: keeps every operand 2-byte so the DVE picks a packed mode;
            # values are already bf16 (read from the strip) so no extra error
            rowtmp = cpool.tile([P, 2 * NB], DT.bfloat16, tag="rowtmp")
            rowacc = cpool.tile([P, NB], DT.float32, tag="rowacc")
            colminT = cpool.tile([P, NB], DT.float32, tag="colminT")
            ones = cpool.tile([P, 1], DT.float32, tag="ones")
            epsc = cpool.tile([P, 1], DT.float32, tag="epsc")

            nc.sync.dma_start(lhsT[:], lhsT_d.ap())
            nc.sync.dma_start(rhs[:], rhs_d.ap())
            nc.sync.dma_start(ident[:], ident_d.ap())
            nc.vector.memset(colacc[:], BIG)
            nc.vector.memset(ones[:], 1.0)
            nc.vector.memset(epsc[:], EPS)

            for bi in range(NB):
                lhs_blk = lhsT[:, bi * P:(bi + 1) * P]
                ps = []
                for h in range(2):
                    pt = ppool.tile([P, H], DT.float32, tag="mm")
                    for q in range(4):
                        off = h * H + q * 512
                        nc.tensor.matmul(
                            pt[:, q * 512:(q + 1) * 512],
                            lhs_blk,
                            rhs[:, off:off + 512],
                            start=True,
                            stop=True,
                        )
                    ps.append(pt)

                sa = spool.tile([P, H], DT.bfloat16, tag="strip")
                sb = spool.tile([P, H], DT.bfloat16, tag="strip")
                nc.scalar.copy(sa[:], ps[0][:])
                nc.scalar.copy(sb[:], ps[1][:])

                # running column-min (per-column over row blocks), bf16 2x
                nc.vector.tensor_tensor(
                    out=colacc[:, 0:H], in0=colacc[:, 0:H], in1=sa[:], op=ALU.min)
                nc.vector.tensor_tensor(
                    out=colacc[:, H:N], in0=colacc[:, H:N], in1=sb[:], op=ALU.min)

                # row-min of each strip in one 4x-mode pass: elementwise out
                # is a dump, the min lands in the accum column
                for h, s in enumerate((sa, sb)):
                    dump = scrpool.tile([P, H], DT.bfloat16, tag="dump")
                    nc.vector.tensor_scalar(
                        out=dump[:], in0=s[:], scalar1=BIG, scalar2=None,
                        op0=ALU.min, op1=ALU.min,
                        accum_out=rowtmp[:, 2 * bi + h:2 * bi + h + 1])

            # combine the two per-strip row-min halves: rowacc = min(even, odd)
            rt = rowtmp[:].rearrange("p (n two) -> p n two", two=2)
            nc.vector.tensor_tensor(
                out=rowacc[:], in0=rt[:, :, 0], in1=rt[:, :, 1], op=ALU.min)

            # column-min partition reduction: transpose 128x128 chunks on PE,
            # 16 chunks per PSUM tile, then one batched 3D min-reduce per tile
            G = 16
            for g in range(NB // G):
                tp = ppool.tile([P, G * P], DT.bfloat16, tag="mm")
                for c in range(G):
                    nc.tensor.transpose(
                        tp[:, c * P:(c + 1) * P],
                        colacc[:, (g * G + c) * P:(g * G + c + 1) * P], ident[:])
                nc.vector.tensor_reduce(
                    out=colminT[:, g * G:(g + 1) * G],
                    in_=tp[:].rearrange("p (n c) -> p n c", c=P),
                    axis=AX.X, op=ALU.min)

            # dist = sqrt(eps + relu(sqmin)); then mean over both directions
            d_row = cpool.tile([P, NB], DT.float32, tag="d_row")
            d_col = cpool.tile([P, NB], DT.float32, tag="d_col")
            nc.vector.tensor_scalar(
                out=d_row[:], in0=rowacc[:], scalar1=0.0, scalar2=None, op0=ALU.max)
            nc.vector.tensor_scalar(
                out=d_col[:], in0=colminT[:], scalar1=0.0, scalar2=None, op0=ALU.max)
            nc.scalar.activation(d_row[:], d_row[:], AF.Sqrt, bias=epsc[:])
            nc.scalar.activation(d_col[:], d_col[:], AF.Sqrt, bias=epsc[:])

            s1 = cpool.tile([P, 1], DT.float32, tag="s1")
            s2 = cpool.tile([P, 1], DT.float32, tag="s2")
            nc.vector.reduce_sum(out=s1[:], in_=d_row[:], axis=AX.X)
            nc.vector.reduce_sum(out=s2[:], in_=d_col[:], axis=AX.X)
            nc.vector.tensor_tensor(out=s1[:], in0=s1[:], in1=s2[:], op=ALU.add)

            pfin = ppool.tile([1, 1], DT.float32, tag="mm")
            nc.tensor.matmul(pfin[:], s1[:], ones[:], start=True, stop=True)
            res = cpool.tile([1, 1], DT.float32, tag="res")
            nc.scalar.mul(res[:], pfin[:], 1.0 / N)
            nc.sync.dma_start(out_d.ap(), res[:])

    return {"lhsT": "lhst_in", "rhs": "rhs_in", "ident": "ident_in",
            "out": "loss_out"}


@functools.lru_cache(maxsize=1)
def build_program():
    nc = bacc.Bacc("TRN2", target_bir_lowering=False, debug=False)
    names = _emit(nc)
    nc.compile()
    return nc, names


def _split(v, levels):
    outs = []
    r = v.astype(np.float64)
    for _ in range(levels):
        s = r.astype(F32).astype(BF16)
        outs.append(s)
        r = r - s.astype(np.float64)
    return outs


# (y-split, x-split) product terms kept; a+b<=2 drops only O(2^-27) terms
_PAIRS = [(0, 0), (0, 1), (1, 0), (1, 1), (0, 2), (2, 0)]


def pack_inputs(x, y):
    """Per-sample packed (lhsT, rhs) bf16 [K, N] operand pair."""
    ys = _split(y, 3)
    xs = _split(x, 3)
    m2x = [(-2.0 * s.astype(F32)).astype(BF16) for s in xs]
    y2 = (y.astype(np.float64) ** 2).sum(1).astype(F32)
    x2 = (x.astype(np.float64) ** 2).sum(1).astype(F32)
    one = np.ones(N, dtype=BF16)
    lrows, rrows = [], []
    for a, b in _PAIRS:
        for c in range(3):
            lrows.append(ys[a][:, c])
            rrows.append(m2x[b][:, c])
    for s in _split(y2, 3):
        lrows.append(s)
        rrows.append(one)
    for s in _split(x2, 3):
        lrows.append(one)
        rrows.append(s)
    lhsT = np.stack(lrows).astype(BF16)
    rhs = np.stack(rrows).astype(BF16)
    assert lhsT.shape == (K, N) and rhs.shape == (K, N)
    return np.ascontiguousarray(lhsT), np.ascontiguousarray(rhs)


def make_in_maps(x, y):
    nc, names = build_program()
    ident = np.eye(P, dtype=BF16)
    in_maps = []
    for b in range(x.shape[0]):
        lhsT, rhs = pack_inputs(np.asarray(x[b]), np.asarray(y[b]))
        in_maps.append({names["lhsT"]: lhsT, names["rhs"]: rhs,
                        names["ident"]: ident})
    return nc, names, in_maps


def run(x, y, trace=False):
    nc, names, in_maps = make_in_maps(x, y)
    res = bass_utils.run_bass_kernel_spmd(
        nc, in_maps, core_ids=list(range(len(in_maps))), trace=trace)
    out = np.array([res.results[b][names["out"]][0, 0]
                    for b in range(len(in_maps))], dtype=F32)
    return out, res


def kernel(x, y):
    out, _ = run(np.asarray(x, dtype=F32), np.asarray(y, dtype=F32))
    return out


# revision 16
# speedup vs baseline: 1.5755x; 1.5755x over previous
"""Chamfer loss Trainium2 kernel.

Per-sample Chamfer loss over (bs=8, n=4096, d=3) point clouds, data-parallel
over the batch axis: one sample per NeuronCore, no cross-core communication.

Math: dist[i,j] = sqrt(eps + relu(||y_i||^2 + ||x_j||^2 - 2 y_i.x_j)).
sqrt(eps + relu(.)) is monotonic, so min-reduce the *squared* matrix and apply
the transform to the reduced 4096-vectors only.

The squared-distance matrix is produced on the TensorEngine as a single K=18
bf16 matmul per tile: y/x are split hi+lo in bf16 (y ~ y0+y1), the squared
norms into three bf16 addends, and all product terms are stacked along the
contraction axis. PSUM accumulates in fp32, giving |sq - exact| ~ 3e-4, i.e.
~1e-5 relative error on the final loss. bf16 streams 1 cycle/row vs fp32's 4.

Per 128-row block (32 of them):
  PE    : 8 matmuls (N=512) into two [128,2048] fp32 PSUM tiles
  ACT   : copies each PSUM tile to an SBUF bf16 strip (frees PSUM, enables
          2x/4x-rate bf16 DVE ops)
  DVE   : running column-min (elementwise bf16 tensor_tensor min into
          colacc[128,4096], 2x mode) and row-min via a bf16 tensor_tensor
          min fold chain 4096->256 (2x mode) + one 1x-rate reduce
          (tensor_scalar's min-accumulate measures 1x on HW, so folds win)
Epilogue: colacc partition-min via PE transpose + DVE min-reduce per 128-col
chunk, then relu/+eps/sqrt on the two [128,32] min matrices, sum-reduce, a
ones-vector matmul for the partition sum, scale by 1/4096.
"""

import os
import sys
import functools

for _p in ("/opt/trn_rl_repo", "/root/.axon_site/_ro/trn_rl_repo"):
    if os.path.isdir(_p) and _p not in sys.path:
        sys.path.insert(0, _p)

import numpy as np
import ml_dtypes

import concourse.bass as bass
import concourse.bacc as bacc
import concourse.mybir as mybir
import concourse.tile as tile
from concourse import bass_utils

BF16 = ml_dtypes.bfloat16
F32 = np.float32

N = 4096          # points per cloud
P = 128           # partitions
NB = N // P       # 32 row blocks
H = 2048          # strip width (half of N), 4 PSUM banks
K = 24            # stacked contraction rows
EPS = 1e-6
BIG = 1e30

AF = mybir.ActivationFunctionType
ALU = mybir.AluOpType
AX = mybir.AxisListType
DT = mybir.dt




def _emit(nc):
    lhsT_d = nc.dram_tensor("lhst_in", [K, N], DT.bfloat16, kind="ExternalInput")
    rhs_d = nc.dram_tensor("rhs_in", [K, N], DT.bfloat16, kind="ExternalInput")
    ident_d = nc.dram_tensor("ident_in", [P, P], DT.bfloat16, kind="ExternalInput")
    out_d = nc.dram_tensor("loss_out", [1, 1], DT.float32, kind="ExternalOutput")

    with tile.TileContext(nc) as tc:
        with (
            tc.tile_pool(name="const", bufs=1) as cpool,
            tc.tile_pool(name="strip", bufs=4) as spool,
            tc.tile_pool(name="scr", bufs=2) as scrpool,
            tc.tile_pool(name="psum", bufs=2, space="PSUM") as ppool,
        ):
            lhsT = cpool.tile([K, N], DT.bfloat16, tag="lhsT")
            rhs = cpool.tile([K, N], DT.bfloat16, tag="rhs")
            ident = cpool.tile([P, P], DT.bfloat16, tag="ident")
            colacc = cpool.tile([P, N], DT.bfloat16, tag="colacc")
            rowacc = cpool.tile([P, NB], DT.float32, tag="rowacc")
            colminT = cpool.tile([P, NB], DT.float32, tag="colminT")
            ones = cpool.tile([P, 1], DT.float32, tag="ones")
            epsc = cpool.tile([P, 1], DT.float32, tag="epsc")

            nc.sync.dma_start(lhsT[:], lhsT_d.ap())
            nc.sync.dma_start(rhs[:], rhs_d.ap())
            nc.sync.dma_start(ident[:], ident_d.ap())
            nc.vector.memset(colacc[:], BIG)
            nc.vector.memset(ones[:], 1.0)
            nc.vector.memset(epsc[:], EPS)

            for bi in range(NB):
                lhs_blk = lhsT[:, bi * P:(bi + 1) * P]
                ps = []
                for h in range(2):
                    pt = ppool.tile([P, H], DT.float32, tag="mm")
                    for q in range(4):
                        off = h * H + q * 512
                        nc.tensor.matmul(
                            pt[:, q * 512:(q + 1) * 512],
                            lhs_blk,
                            rhs[:, off:off + 512],
                            start=True,
                            stop=True,
                        )
                    ps.append(pt)

                sa = spool.tile([P, H], DT.bfloat16, tag="strip")
                sb = spool.tile([P, H], DT.bfloat16, tag="strip")
                nc.scalar.copy(sa[:], ps[0][:])
                nc.scalar.copy(sb[:], ps[1][:])

                # running column-min (per-column over row blocks), bf16 2x
                nc.vector.tensor_tensor(
                    out=colacc[:, 0:H], in0=colacc[:, 0:H], in1=sa[:], op=ALU.min)
                nc.vector.tensor_tensor(
                    out=colacc[:, H:N], in0=colacc[:, H:N], in1=sb[:], op=ALU.min)

                # row-min: bf16 pairwise-min folds at 2x, then a small reduce
                f1 = scrpool.tile([P, H], DT.bfloat16, tag="f1")
                f2 = scrpool.tile([P, H // 2], DT.bfloat16, tag="f2")
                f3 = scrpool.tile([P, H // 4], DT.bfloat16, tag="f3")
                f4 = scrpool.tile([P, H // 8], DT.bfloat16, tag="f4")
                nc.vector.tensor_tensor(out=f1[:], in0=sa[:], in1=sb[:], op=ALU.min)
                nc.vector.tensor_tensor(
                    out=f2[:], in0=f1[:, 0:H // 2], in1=f1[:, H // 2:H], op=ALU.min)
                nc.vector.tensor_tensor(
                    out=f3[:], in0=f2[:, 0:H // 4], in1=f2[:, H // 4:H // 2],
                    op=ALU.min)
                nc.vector.tensor_tensor(
                    out=f4[:], in0=f3[:, 0:H // 8], in1=f3[:, H // 8:H // 4],
                    op=ALU.min)
                nc.vector.tensor_reduce(
                    out=rowacc[:, bi:bi + 1], in_=f4[:], axis=AX.X, op=ALU.min)

            # column-min partition reduction: transpose 128x128 chunks on PE,
            # 16 chunks per PSUM tile, then one batched 3D min-reduce per tile
            G = 16
            for g in range(NB // G):
                tp = ppool.tile([P, G * P], DT.bfloat16, tag="mm")
                for c in range(G):
                    nc.tensor.transpose(
                        tp[:, c * P:(c + 1) * P],
                        colacc[:, (g * G + c) * P:(g * G + c + 1) * P], ident[:])
                nc.vector.tensor_reduce(
                    out=colminT[:, g * G:(g + 1) * G],
                    in_=tp[:].rearrange("p (n c) -> p n c", c=P),
                    axis=AX.X, op=ALU.min)

            # dist = sqrt(eps + relu(sqmin)); then mean over both directions
            d_row = cpool.tile([P, NB], DT.float32, tag="d_row")
            d_col = cpool.tile([P, NB], DT.float32, tag="d_col")
            nc.vector.tensor_scalar(
                out=d_row[:], in0=rowacc[:], scalar1=0.0, scalar2=None, op0=ALU.max)
            nc.vector.tensor_scalar(
                out=d_col[:], in0=colminT[:], scalar1=0.0, scalar2=None, op0=ALU.max)
            nc.scalar.activation(d_row[:], d_row[:], AF.Sqrt, bias=epsc[:])
            nc.scalar.activation(d_col[:], d_col[:], AF.Sqrt, bias=epsc[:])

            s1 = cpool.tile([P, 1], DT.float32, tag="s1")
            s2 = cpool.tile([P, 1], DT.float32, tag="s2")
            nc.vector.reduce_sum(out=s1[:], in_=d_row[:], axis=AX.X)
            nc.vector.reduce_sum(out=s2[:], in_=d_col[:], axis=AX.X)
            nc.vector.tensor_tensor(out=s1[:], in0=s1[:], in1=s2[:], op=ALU.add)

            pfin = ppool.tile([1, 1], DT.float32, tag="mm")
            nc.tensor.matmul(pfin[:], s1[:], ones[:], start=True, stop=True)
            res = cpool.tile([1, 1], DT.float32, tag="res")
            nc.scalar.mul(res[:], pfin[:], 1.0 / N)
            nc.sync.dma_start(out_d.ap(), res[:])

    return {"lhsT": "lhst_in", "rhs": "rhs_in", "ident": "ident_in",
            "out": "loss_out"}


@functools.lru_cache(maxsize=1)
def build_program():
    nc = bacc.Bacc("TRN2", target_bir_lowering=False, debug=False)
    names = _emit(nc)
    nc.compile()
    return nc, names


def _split(v, levels):
    outs = []
    r = v.astype(np.float64)
    for _ in range(levels):
        s = r.astype(F32).astype(BF16)
        outs.append(s)
        r = r - s.astype(np.float64)
    return outs


# (y-split, x-split) product terms kept; a+b<=2 drops only O(2^-27) terms
_PAIRS = [(0, 0), (0, 1), (1, 0), (1, 1), (0, 2), (2, 0)]


def pack_inputs(x, y):
    """Per-sample packed (lhsT, rhs) bf16 [K, N] operand pair."""
    ys = _split(y, 3)
    xs = _split(x, 3)
    m2x = [(-2.0 * s.astype(F32)).astype(BF16) for s in xs]
    y2 = (y.astype(np.float64) ** 2).sum(1).astype(F32)
    x2 = (x.astype(np.float64) ** 2).sum(1).astype(F32)
    one = np.ones(N, dtype=BF16)
    lrows, rrows = [], []
    for a, b in _PAIRS:
        for c in range(3):
            lrows.append(ys[a][:, c])
            rrows.append(m2x[b][:, c])
    for s in _split(y2, 3):
        lrows.append(s)
        rrows.append(one)
    for s in _split(x2, 3):
        lrows.append(one)
        rrows.append(s)
    lhsT = np.stack(lrows).astype(BF16)
    rhs = np.stack(rrows).astype(BF16)
    assert lhsT.shape == (K, N) and rhs.shape == (K, N)
    return np.ascontiguousarray(lhsT), np.ascontiguousarray(rhs)


def make_in_maps(x, y):
    nc, names = build_program()
    ident = np.eye(P, dtype=BF16)
    in_maps = []
    for b in range(x.shape[0]):
        lhsT, rhs = pack_inputs(np.asarray(x[b]), np.asarray(y[b]))
        in_maps.append({names["lhsT"]: lhsT, names["rhs"]: rhs,
                        names["ident"]: ident})
    return nc, names, in_maps


def run(x, y, trace=False):
    nc, names, in_maps = make_in_maps(x, y)
    res = bass_utils.run_bass_kernel_spmd(
        nc, in_maps, core_ids=list(range(len(in_maps))), trace=trace)
    out = np.array([res.results[b][names["out"]][0, 0]
                    for b in range(len(in_maps))], dtype=F32)
    return out, res


def kernel(x, y):
    out, _ = run(np.asarray(x, dtype=F32), np.asarray(y, dtype=F32))
    return out
